# revision 7
# baseline (speedup 1.0000x reference)
"""Chamfer distance loss kernel for Trainium2 (8 NeuronCores, SPMD).

Problem: x1 [16, 4096, 3], x2 [16, 4096, 3] ->
    chamfer[b] = mean_i min_j ||x1[b,i]-x2[b,j]||^2 + mean_j min_i ||...||^2

Strategy:
  - Data-parallel over batch: 2 batches per core.
  - Distance embedding (K=5 matmul): u_i = [x1_i, |x1_i|^2, 1],
    v_j = [-2*x2_j, 1, |x2_j|^2]  =>  u_i . v_j = ||x1_i - x2_j||^2.
  - Pair-min trick: distances are linear in the opposite side's embedding, so
    with host-prepared vs_k = (v_{2k}+v_{2k+1})/2 and vd_k = (v_{2k}-v_{2k+1})/2
    the PE produces P = u.vs and M = u.vd, and
    min(d_{i,2k}, d_{i,2k+1}) = P - |M|.
    ScalarE computes |M| (PSUM->SBUF); one VectorE tensor_tensor_reduce per
    span computes (P - |M|) elementwise AND min-reduces it into a row-min
    slot. Each distance crosses the DVE exactly once per reduction direction,
    at half the element count thanks to pairing.
  - Two streams per batch: stream A (lhsT = u-chunks, rhs = vs/vd) yields
    min over j for each i; stream B (lhsT = v-chunks, rhs = us/ud) yields
    min over i for each j. No partition-axis reductions needed.
  - Host folds the small [128, 64] slot tensors into the final [16] means.
"""

import sys

for _p in ("/opt/trn_rl_repo",):
    if _p not in sys.path:
        sys.path.insert(0, _p)

import numpy as np

B, N, M = 16, 4096, 4096
NCORES = 8
BPC = B // NCORES  # batches per core
K = 5  # embedded contraction dim
P = 128  # partitions
SPAN = 1024  # pair-elements per DVE consume span
NSPAN = (M // 2) // SPAN  # spans per chunk (pairs per row = 2048)
MMBLK = 512  # matmul free dim (1 PSUM bank)
NCHUNK = N // P  # 32 chunks of the partition-side
BIG = 3.0e38

_built = None


def _build_nc():
    import concourse.bacc as bacc
    import concourse.mybir as mybir
    import concourse.tile as tile

    f32 = mybir.dt.float32
    mn = mybir.AluOpType.min
    sub = mybir.AluOpType.subtract

    nc = bacc.Bacc(
        "TRN2", target_bir_lowering=False, debug=False, num_devices=NCORES
    )
    # full embeddings (lhsT side) and paired embeddings (rhs side)
    u_ext = nc.dram_tensor("u", [BPC, K, N], f32, kind="ExternalInput").ap()
    v_ext = nc.dram_tensor("v", [BPC, K, M], f32, kind="ExternalInput").ap()
    vs_ext = nc.dram_tensor("vs", [BPC, K, M // 2], f32, kind="ExternalInput").ap()
    vd_ext = nc.dram_tensor("vd", [BPC, K, M // 2], f32, kind="ExternalInput").ap()
    us_ext = nc.dram_tensor("us", [BPC, K, N // 2], f32, kind="ExternalInput").ap()
    ud_ext = nc.dram_tensor("ud", [BPC, K, N // 2], f32, kind="ExternalInput").ap()
    ra_ext = nc.dram_tensor(
        "rowacc", [BPC, P, NCHUNK * NSPAN], f32, kind="ExternalOutput"
    ).ap()
    ca_ext = nc.dram_tensor(
        "colacc", [BPC, P, NCHUNK * NSPAN], f32, kind="ExternalOutput"
    ).ap()

    with tile.TileContext(nc) as tc:
        with (
            tc.tile_pool(name="const", bufs=1) as cpool,
            tc.tile_pool(name="uv", bufs=1) as uvpool,
            tc.tile_pool(name="acc", bufs=1) as apool,
            tc.tile_pool(name="work", bufs=3) as wpool,
            tc.tile_pool(name="psum", bufs=2, space="PSUM") as ppool,
        ):
            from concourse import masks

            negI = cpool.tile([P, P], f32, tag="negI", name="negI")
            masks.make_identity(nc, negI[:])
            nc.scalar.mul(negI[:], negI[:], -1.0)
            for b in range(BPC):
                U = uvpool.tile([K, N], f32, tag=f"u{b}", name=f"u{b}")
                V = uvpool.tile([K, M], f32, tag=f"v{b}", name=f"v{b}")
                Vs = uvpool.tile([K, M // 2], f32, tag=f"vs{b}", name=f"vs{b}")
                Vd = uvpool.tile([K, M // 2], f32, tag=f"vd{b}", name=f"vd{b}")
                Us = uvpool.tile([K, N // 2], f32, tag=f"us{b}", name=f"us{b}")
                Ud = uvpool.tile([K, N // 2], f32, tag=f"ud{b}", name=f"ud{b}")
                nc.sync.dma_start(U[:], u_ext[b])
                nc.sync.dma_start(V[:], v_ext[b])
                nc.sync.dma_start(Vs[:], vs_ext[b])
                nc.sync.dma_start(Vd[:], vd_ext[b])
                nc.sync.dma_start(Us[:], us_ext[b])
                nc.sync.dma_start(Ud[:], ud_ext[b])
                rowacc = apool.tile(
                    [P, NCHUNK * NSPAN], f32, tag=f"ra{b}", name=f"ra{b}"
                )
                colacc = apool.tile(
                    [P, NCHUNK * NSPAN], f32, tag=f"ca{b}", name=f"ca{b}"
                )
                for lhsT, rs, rd, outslots in (
                    (U, Vs, Vd, rowacc),
                    (V, Us, Ud, colacc),
                ):
                    for c in range(NCHUNK):
                        lw = lhsT[:, c * P : (c + 1) * P]
                        for sp in range(NSPAN):
                            # pm layout: [:, :SPAN] = P pairsum, [:, SPAN:] = M pairdiff
                            pm = ppool.tile([P, 2 * SPAN], f32, tag="pm", name="pm")
                            # pair-diff matmuls (M half)
                            for h in range(SPAN // MMBLK):
                                j0 = sp * SPAN + h * MMBLK
                                nc.tensor.matmul(
                                    pm[:, SPAN + h * MMBLK : SPAN + (h + 1) * MMBLK],
                                    lw,
                                    rd[:, j0 : j0 + MMBLK],
                                    start=True,
                                    stop=True,
                                )
                            # pair-sum matmuls (P half), accumulation group open
                            for h in range(SPAN // MMBLK):
                                j0 = sp * SPAN + h * MMBLK
                                nc.tensor.matmul(
                                    pm[:, h * MMBLK : (h + 1) * MMBLK],
                                    lw,
                                    rs[:, j0 : j0 + MMBLK],
                                    start=True,
                                    stop=False,
                                )
                            absm = wpool.tile([P, SPAN], f32, tag="absm", name="absm")
                            nc.scalar.activation(
                                absm[:],
                                pm[:, SPAN : 2 * SPAN],
                                mybir.ActivationFunctionType.Abs,
                            )
                            # accumulate -|M| onto the P half: pairmin = P - |M|
                            for h in range(SPAN // MMBLK):
                                nc.tensor.matmul(
                                    pm[:, h * MMBLK : (h + 1) * MMBLK],
                                    negI[:],
                                    absm[:, h * MMBLK : (h + 1) * MMBLK],
                                    start=False,
                                    stop=True,
                                )
                            idx = c * NSPAN + sp
                            nc.vector.tensor_reduce(
                                out=outslots[:, idx : idx + 1],
                                in_=pm[:, 0:SPAN],
                                axis=mybir.AxisListType.X,
                                op=mn,
                            )
                nc.sync.dma_start(ra_ext[b], rowacc[:])
                nc.sync.dma_start(ca_ext[b], colacc[:])
    nc.compile()
    return nc


def _prep_in_maps(x1: np.ndarray, x2: np.ndarray):
    x1 = np.asarray(x1, dtype=np.float32)
    x2 = np.asarray(x2, dtype=np.float32)
    n1 = (x1 * x1).sum(-1)  # [B, N]
    n2 = (x2 * x2).sum(-1)  # [B, M]
    u_all = np.concatenate(
        [x1.transpose(0, 2, 1), n1[:, None, :], np.ones((B, 1, N), np.float32)],
        axis=1,
    )  # [B, 5, N]
    v_all = np.concatenate(
        [-2.0 * x2.transpose(0, 2, 1), np.ones((B, 1, M), np.float32), n2[:, None, :]],
        axis=1,
    )  # [B, 5, M]
    vs = (v_all[:, :, 0::2] + v_all[:, :, 1::2]) * np.float32(0.5)
    vd = (v_all[:, :, 0::2] - v_all[:, :, 1::2]) * np.float32(0.5)
    us = (u_all[:, :, 0::2] + u_all[:, :, 1::2]) * np.float32(0.5)
    ud = (u_all[:, :, 0::2] - u_all[:, :, 1::2]) * np.float32(0.5)
    c = np.ascontiguousarray
    return [
        {
            "u": c(u_all[i * BPC : (i + 1) * BPC]),
            "v": c(v_all[i * BPC : (i + 1) * BPC]),
            "vs": c(vs[i * BPC : (i + 1) * BPC]),
            "vd": c(vd[i * BPC : (i + 1) * BPC]),
            "us": c(us[i * BPC : (i + 1) * BPC]),
            "ud": c(ud[i * BPC : (i + 1) * BPC]),
        }
        for i in range(NCORES)
    ]


def _run(in_maps, trace=False):
    from concourse.bass_utils import run_bass_kernel_spmd

    global _built
    if _built is None:
        _built = _build_nc()
    return run_bass_kernel_spmd(
        _built, in_maps, list(range(NCORES)), trace=trace
    )


def _postprocess(results):
    out = np.empty((B,), np.float32)
    for c in range(NCORES):
        ra = results[c]["rowacc"]  # [BPC, 128, NCHUNK*NSPAN]
        ca = results[c]["colacc"]
        for b in range(BPC):
            m1 = ra[b].reshape(P, NCHUNK, NSPAN).min(axis=2)
            m2 = ca[b].reshape(P, NCHUNK, NSPAN).min(axis=2)
            out[c * BPC + b] = np.float32(
                m1.mean(dtype=np.float64) + m2.mean(dtype=np.float64)
            )
    return out


def kernel(x1: np.ndarray, x2: np.ndarray) -> np.ndarray:
    res = _run(_prep_in_maps(x1, x2))
    return _postprocess(res.results)


# revision 11
# speedup vs baseline: 218.1079x; 218.1079x over previous
"""Chamfer distance loss kernel for Trainium2 (8 NeuronCores, SPMD).

Problem: x1 [16, 4096, 3], x2 [16, 4096, 3] ->
    chamfer[b] = mean_i min_j ||x1[b,i]-x2[b,j]||^2 + mean_j min_i ||...||^2

Strategy:
  - Data-parallel over batch: 2 batches per core.
  - Distance embedding (K=5 matmul): u_i = [x1_i, |x1_i|^2, 1],
    v_j = [-2*x2_j, 1, |x2_j|^2]  =>  u_i . v_j = ||x1_i - x2_j||^2.
  - Pair-min trick: distances are linear in the opposite side's embedding, so
    with host-prepared vs_k = (v_{2k}+v_{2k+1})/2 and vd_k = (v_{2k}-v_{2k+1})/2
    the PE produces P = u.vs and M = u.vd, and
    min(d_{i,2k}, d_{i,2k+1}) = P - |M|.
    ScalarE computes |M| (PSUM->SBUF); one VectorE tensor_tensor_reduce per
    span computes (P - |M|) elementwise AND min-reduces it into a row-min
    slot. Each distance crosses the DVE exactly once per reduction direction,
    at half the element count thanks to pairing.
  - Two streams per batch: stream A (lhsT = u-chunks, rhs = vs/vd) yields
    min over j for each i; stream B (lhsT = v-chunks, rhs = us/ud) yields
    min over i for each j. No partition-axis reductions needed.
  - Host folds the small [128, 64] slot tensors into the final [16] means.
"""

import sys

for _p in ("/opt/trn_rl_repo",):
    if _p not in sys.path:
        sys.path.insert(0, _p)

import numpy as np

B, N, M = 16, 4096, 4096
NCORES = 8
BPC = B // NCORES  # batches per core
K = 5  # embedded contraction dim
P = 128  # partitions
SPAN = 1024  # pair-elements per DVE consume span
NSPAN = (M // 2) // SPAN  # spans per chunk (pairs per row = 2048)
MMBLK = 512  # matmul free dim (1 PSUM bank)
NCHUNK = N // P  # 32 chunks of the partition-side
BIG = 3.0e38

_built = {}


def _build_nc(repeat=1):
    import concourse.bacc as bacc
    import concourse.mybir as mybir
    import concourse.tile as tile

    f32 = mybir.dt.float32
    mn = mybir.AluOpType.min
    sub = mybir.AluOpType.subtract

    nc = bacc.Bacc(
        "TRN2", target_bir_lowering=False, debug=False, num_devices=NCORES
    )
    # full embeddings (lhsT side) and paired embeddings (rhs side)
    u_ext = nc.dram_tensor("u", [BPC, K, N], f32, kind="ExternalInput").ap()
    v_ext = nc.dram_tensor("v", [BPC, K, M], f32, kind="ExternalInput").ap()
    vs_ext = nc.dram_tensor("vs", [BPC, K, M // 2], f32, kind="ExternalInput").ap()
    vd_ext = nc.dram_tensor("vd", [BPC, K, M // 2], f32, kind="ExternalInput").ap()
    us_ext = nc.dram_tensor("us", [BPC, K, N // 2], f32, kind="ExternalInput").ap()
    ud_ext = nc.dram_tensor("ud", [BPC, K, N // 2], f32, kind="ExternalInput").ap()
    ra_ext = nc.dram_tensor(
        "rowacc", [BPC, P, NCHUNK * NSPAN], f32, kind="ExternalOutput"
    ).ap()
    ca_ext = nc.dram_tensor(
        "colacc", [BPC, P, NCHUNK * NSPAN], f32, kind="ExternalOutput"
    ).ap()

    with tile.TileContext(nc) as tc:
        with (
            tc.tile_pool(name="const", bufs=1) as cpool,
            tc.tile_pool(name="uv", bufs=1) as uvpool,
            tc.tile_pool(name="acc", bufs=1) as apool,
            tc.tile_pool(name="work", bufs=3) as wpool,
            tc.tile_pool(name="psum", bufs=2, space="PSUM") as ppool,
        ):
            from concourse import masks

            negI = cpool.tile([P, P], f32, tag="negI", name="negI")
            masks.make_identity(nc, negI[:])
            nc.scalar.mul(negI[:], negI[:], -1.0)

            def body():
                _body(nc, tc, mybir, uvpool, apool, wpool, ppool, negI,
                      (u_ext, v_ext, vs_ext, vd_ext, us_ext, ud_ext),
                      (ra_ext, ca_ext))

            if repeat == 1:
                body()
            else:
                with tc.For_i(0, repeat, 1):
                    body()
    nc.compile()
    return nc


def _body(nc, tc, mybir, uvpool, apool, wpool, ppool, negI, ins, outs):
    f32 = mybir.dt.float32
    mn = mybir.AluOpType.min
    u_ext, v_ext, vs_ext, vd_ext, us_ext, ud_ext = ins
    ra_ext, ca_ext = outs
    if True:
        if True:
            for b in range(BPC):
                U = uvpool.tile([K, N], f32, tag=f"u{b}", name=f"u{b}")
                V = uvpool.tile([K, M], f32, tag=f"v{b}", name=f"v{b}")
                Vs = uvpool.tile([K, M // 2], f32, tag=f"vs{b}", name=f"vs{b}")
                Vd = uvpool.tile([K, M // 2], f32, tag=f"vd{b}", name=f"vd{b}")
                Us = uvpool.tile([K, N // 2], f32, tag=f"us{b}", name=f"us{b}")
                Ud = uvpool.tile([K, N // 2], f32, tag=f"ud{b}", name=f"ud{b}")
                nc.sync.dma_start(U[:], u_ext[b])
                nc.sync.dma_start(V[:], v_ext[b])
                nc.sync.dma_start(Vs[:], vs_ext[b])
                nc.sync.dma_start(Vd[:], vd_ext[b])
                nc.sync.dma_start(Us[:], us_ext[b])
                nc.sync.dma_start(Ud[:], ud_ext[b])
                rowacc = apool.tile(
                    [P, NCHUNK * NSPAN], f32, tag=f"ra{b}", name=f"ra{b}"
                )
                colacc = apool.tile(
                    [P, NCHUNK * NSPAN], f32, tag=f"ca{b}", name=f"ca{b}"
                )
                for lhsT, rs, rd, outslots in (
                    (U, Vs, Vd, rowacc),
                    (V, Us, Ud, colacc),
                ):
                    for c in range(NCHUNK):
                        lw = lhsT[:, c * P : (c + 1) * P]
                        for sp in range(NSPAN):
                            # pm layout: [:, :SPAN] = P pairsum, [:, SPAN:] = M pairdiff
                            pm = ppool.tile([P, 2 * SPAN], f32, tag="pm", name="pm")
                            # pair-diff matmuls (M half)
                            for h in range(SPAN // MMBLK):
                                j0 = sp * SPAN + h * MMBLK
                                nc.tensor.matmul(
                                    pm[:, SPAN + h * MMBLK : SPAN + (h + 1) * MMBLK],
                                    lw,
                                    rd[:, j0 : j0 + MMBLK],
                                    start=True,
                                    stop=True,
                                )
                            # pair-sum matmuls (P half), accumulation group open
                            for h in range(SPAN // MMBLK):
                                j0 = sp * SPAN + h * MMBLK
                                nc.tensor.matmul(
                                    pm[:, h * MMBLK : (h + 1) * MMBLK],
                                    lw,
                                    rs[:, j0 : j0 + MMBLK],
                                    start=True,
                                    stop=False,
                                )
                            absm = wpool.tile([P, SPAN], f32, tag="absm", name="absm")
                            nc.scalar.activation(
                                absm[:],
                                pm[:, SPAN : 2 * SPAN],
                                mybir.ActivationFunctionType.Abs,
                            )
                            # accumulate -|M| onto the P half: pairmin = P - |M|
                            for h in range(SPAN // MMBLK):
                                nc.tensor.matmul(
                                    pm[:, h * MMBLK : (h + 1) * MMBLK],
                                    negI[:],
                                    absm[:, h * MMBLK : (h + 1) * MMBLK],
                                    start=False,
                                    stop=True,
                                )
                            idx = c * NSPAN + sp
                            nc.vector.tensor_reduce(
                                out=outslots[:, idx : idx + 1],
                                in_=pm[:, 0:SPAN],
                                axis=mybir.AxisListType.X,
                                op=mn,
                            )
                nc.sync.dma_start(ra_ext[b], rowacc[:])
                nc.sync.dma_start(ca_ext[b], colacc[:])


def _prep_in_maps(x1: np.ndarray, x2: np.ndarray):
    x1 = np.asarray(x1, dtype=np.float32)
    x2 = np.asarray(x2, dtype=np.float32)
    n1 = (x1 * x1).sum(-1)  # [B, N]
    n2 = (x2 * x2).sum(-1)  # [B, M]
    u_all = np.concatenate(
        [x1.transpose(0, 2, 1), n1[:, None, :], np.ones((B, 1, N), np.float32)],
        axis=1,
    )  # [B, 5, N]
    v_all = np.concatenate(
        [-2.0 * x2.transpose(0, 2, 1), np.ones((B, 1, M), np.float32), n2[:, None, :]],
        axis=1,
    )  # [B, 5, M]
    vs = (v_all[:, :, 0::2] + v_all[:, :, 1::2]) * np.float32(0.5)
    vd = (v_all[:, :, 0::2] - v_all[:, :, 1::2]) * np.float32(0.5)
    us = (u_all[:, :, 0::2] + u_all[:, :, 1::2]) * np.float32(0.5)
    ud = (u_all[:, :, 0::2] - u_all[:, :, 1::2]) * np.float32(0.5)
    c = np.ascontiguousarray
    return [
        {
            "u": c(u_all[i * BPC : (i + 1) * BPC]),
            "v": c(v_all[i * BPC : (i + 1) * BPC]),
            "vs": c(vs[i * BPC : (i + 1) * BPC]),
            "vd": c(vd[i * BPC : (i + 1) * BPC]),
            "us": c(us[i * BPC : (i + 1) * BPC]),
            "ud": c(ud[i * BPC : (i + 1) * BPC]),
        }
        for i in range(NCORES)
    ]


def _run(in_maps, trace=False, repeat=1):
    from concourse.bass_utils import run_bass_kernel_spmd

    if repeat not in _built:
        _built[repeat] = _build_nc(repeat)
    return run_bass_kernel_spmd(
        _built[repeat], in_maps, list(range(NCORES)), trace=trace
    )


def _postprocess(results):
    out = np.empty((B,), np.float32)
    for c in range(NCORES):
        ra = results[c]["rowacc"]  # [BPC, 128, NCHUNK*NSPAN]
        ca = results[c]["colacc"]
        for b in range(BPC):
            m1 = ra[b].reshape(P, NCHUNK, NSPAN).min(axis=2)
            m2 = ca[b].reshape(P, NCHUNK, NSPAN).min(axis=2)
            out[c * BPC + b] = np.float32(
                m1.mean(dtype=np.float64) + m2.mean(dtype=np.float64)
            )
    return out


def kernel(x1: np.ndarray, x2: np.ndarray) -> np.ndarray:
    res = _run(_prep_in_maps(x1, x2))
    return _postprocess(res.results)


# revision 13
# speedup vs baseline: 759.0188x; 3.4800x over previous
"""Chamfer distance loss kernel for Trainium2 (8 NeuronCores, SPMD).

Problem: x1 [16, 4096, 3], x2 [16, 4096, 3] ->
    chamfer[b] = mean_i min_j ||x1[b,i]-x2[b,j]||^2 + mean_j min_i ||...||^2

Strategy:
  - Data-parallel over batch: 2 batches per core.
  - Distance embedding: u_i = [x1_i, |x1_i|^2, 1], v_j = [-2*x2_j, 1, |x2_j|^2]
    => u_i . v_j = ||x1_i - x2_j||^2, so a [K, 128] x [K, FD] matmul produces a
    128 x FD block of the distance matrix in PSUM.
  - fp32 matmuls stream at 1/4 rate on the PE, so inputs are split into 3
    bfloat16 components (hi/mid/lo) and the K dim carries the full 3x3 outer
    product (K = 45): exactly (uh+um+ul).(vh+vm+vl), reproducing the fp32 dot
    to ~fp32 accuracy at full bf16 streaming rate.
  - Two streams per batch: stream A (lhsT = u-chunks, rhs = v-embeddings)
    yields min over j for each i via a free-axis min reduce; stream B
    (lhsT = v-chunks, rhs = u-embeddings) yields min over i for each j the
    same way. No partition-axis reductions, no cross-tile state.
  - Host folds the small [128, 64] slot tensors into the final [16] means.
"""

import sys

for _p in ("/opt/trn_rl_repo",):
    if _p not in sys.path:
        sys.path.insert(0, _p)

import ml_dtypes
import numpy as np

B, N, M = 16, 4096, 4096
NCORES = 8
BPC = B // NCORES  # batches per core
K = 5  # embedding dim; K3 = 3 bf16 splits x 3 = 45 matmul contraction
K3 = 9 * K
P = 128  # partitions
SPAN = 2048  # distance elements per DVE reduce span (4 PSUM banks)
NSPAN = M // SPAN  # spans per chunk
MMBLK = 512  # matmul free dim (1 PSUM bank)
NCHUNK = N // P  # 32 chunks of the partition-side
BIG = 3.0e38

_built = {}


def _build_nc(repeat=1):
    import concourse.bacc as bacc
    import concourse.mybir as mybir
    import concourse.tile as tile

    f32 = mybir.dt.float32
    bf16 = mybir.dt.bfloat16

    nc = bacc.Bacc(
        "TRN2", target_bir_lowering=False, debug=False, num_devices=NCORES
    )
    # Stream A: lhsT = ut3 (u tiled x3), rhs = vr3 (v repeated x3)
    # Stream B: lhsT = vt3,              rhs = ur3
    ut3_ext = nc.dram_tensor("ut3", [BPC, K3, N], bf16, kind="ExternalInput").ap()
    vr3_ext = nc.dram_tensor("vr3", [BPC, K3, M], bf16, kind="ExternalInput").ap()
    vt3_ext = nc.dram_tensor("vt3", [BPC, K3, M], bf16, kind="ExternalInput").ap()
    ur3_ext = nc.dram_tensor("ur3", [BPC, K3, N], bf16, kind="ExternalInput").ap()
    ra_ext = nc.dram_tensor(
        "rowacc", [BPC, P, NCHUNK * NSPAN], f32, kind="ExternalOutput"
    ).ap()
    ca_ext = nc.dram_tensor(
        "colacc", [BPC, P, NCHUNK * NSPAN], f32, kind="ExternalOutput"
    ).ap()

    with tile.TileContext(nc) as tc:
        with (
            tc.tile_pool(name="uv", bufs=1) as uvpool,
            tc.tile_pool(name="acc", bufs=1) as apool,
            tc.tile_pool(name="psum", bufs=2, space="PSUM") as ppool,
        ):

            def body():
                _body(
                    nc, mybir, uvpool, apool, ppool,
                    (ut3_ext, vr3_ext, vt3_ext, ur3_ext),
                    (ra_ext, ca_ext),
                )

            if repeat == 1:
                body()
            else:
                with tc.For_i(0, repeat, 1):
                    body()
    nc.compile()
    return nc


def _body(nc, mybir, uvpool, apool, ppool, ins, outs):
    f32 = mybir.dt.float32
    bf16 = mybir.dt.bfloat16
    mn = mybir.AluOpType.min
    ut3_ext, vr3_ext, vt3_ext, ur3_ext = ins
    ra_ext, ca_ext = outs
    for b in range(BPC):
        Ut = uvpool.tile([K3, N], bf16, tag=f"ut{b}", name=f"ut{b}")
        Vr = uvpool.tile([K3, M], bf16, tag=f"vr{b}", name=f"vr{b}")
        Vt = uvpool.tile([K3, M], bf16, tag=f"vt{b}", name=f"vt{b}")
        Ur = uvpool.tile([K3, N], bf16, tag=f"ur{b}", name=f"ur{b}")
        nc.sync.dma_start(Ut[:], ut3_ext[b])
        nc.sync.dma_start(Vr[:], vr3_ext[b])
        nc.sync.dma_start(Vt[:], vt3_ext[b])
        nc.sync.dma_start(Ur[:], ur3_ext[b])
        rowacc = apool.tile([P, NCHUNK * NSPAN], f32, tag=f"ra{b}", name=f"ra{b}")
        colacc = apool.tile([P, NCHUNK * NSPAN], f32, tag=f"ca{b}", name=f"ca{b}")
        for lhsT, rhs, outslots in (
            (Ut, Vr, rowacc),
            (Vt, Ur, colacc),
        ):
            for c in range(NCHUNK):
                lw = lhsT[:, c * P : (c + 1) * P]
                for sp in range(NSPAN):
                    dist = ppool.tile([P, SPAN], f32, tag="dist", name="dist")
                    for h in range(SPAN // MMBLK):
                        j0 = sp * SPAN + h * MMBLK
                        nc.tensor.matmul(
                            dist[:, h * MMBLK : (h + 1) * MMBLK],
                            lw,
                            rhs[:, j0 : j0 + MMBLK],
                            start=True,
                            stop=True,
                        )
                    idx = c * NSPAN + sp
                    nc.vector.tensor_reduce(
                        out=outslots[:, idx : idx + 1],
                        in_=dist[:],
                        axis=mybir.AxisListType.X,
                        op=mn,
                    )
        nc.sync.dma_start(ra_ext[b], rowacc[:])
        nc.sync.dma_start(ca_ext[b], colacc[:])


def _split3(a):
    """Split fp32 array into 3 bf16 components summing to ~a (fp32 accurate)."""
    a = np.asarray(a, np.float32)
    h = a.astype(ml_dtypes.bfloat16)
    r = a - h.astype(np.float32)
    m = r.astype(ml_dtypes.bfloat16)
    l = (r - m.astype(np.float32)).astype(ml_dtypes.bfloat16)
    return h, m, l


def _prep_in_maps(x1: np.ndarray, x2: np.ndarray):
    x1 = np.asarray(x1, dtype=np.float32)
    x2 = np.asarray(x2, dtype=np.float32)
    # center the clouds (chamfer is translation invariant; shrinks magnitudes
    # so the bf16-split dot keeps more effective precision)
    ctr = 0.5 * (x1.mean(axis=(1,), keepdims=True) + x2.mean(axis=(1,), keepdims=True))
    x1c = x1 - ctr
    x2c = x2 - ctr
    n1 = (x1c.astype(np.float64) ** 2).sum(-1).astype(np.float32)  # [B, N]
    n2 = (x2c.astype(np.float64) ** 2).sum(-1).astype(np.float32)  # [B, M]
    u_all = np.concatenate(
        [x1c.transpose(0, 2, 1), n1[:, None, :], np.ones((B, 1, N), np.float32)],
        axis=1,
    )  # [B, 5, N]
    v_all = np.concatenate(
        [
            -2.0 * x2c.transpose(0, 2, 1),
            np.ones((B, 1, M), np.float32),
            n2[:, None, :],
        ],
        axis=1,
    )  # [B, 5, M]
    uh, um, ul = _split3(u_all)
    vh, vm, vl = _split3(v_all)
    usplit = np.concatenate([uh, um, ul], axis=1)  # [B, 15, N]
    vsplit = np.concatenate([vh, vm, vl], axis=1)  # [B, 15, M]
    ut3 = np.tile(usplit, (1, 3, 1))  # [B, 45, N]  (uh um ul uh um ul uh um ul)
    vr3 = np.concatenate([vh, vh, vh, vm, vm, vm, vl, vl, vl], axis=1)  # [B, 45, M]
    vt3 = np.tile(vsplit, (1, 3, 1))
    ur3 = np.concatenate([uh, uh, uh, um, um, um, ul, ul, ul], axis=1)
    c = np.ascontiguousarray
    return [
        {
            "ut3": c(ut3[i * BPC : (i + 1) * BPC]),
            "vr3": c(vr3[i * BPC : (i + 1) * BPC]),
            "vt3": c(vt3[i * BPC : (i + 1) * BPC]),
            "ur3": c(ur3[i * BPC : (i + 1) * BPC]),
        }
        for i in range(NCORES)
    ]


def _run(in_maps, trace=False, repeat=1):
    from concourse.bass_utils import run_bass_kernel_spmd

    if repeat not in _built:
        _built[repeat] = _build_nc(repeat)
    return run_bass_kernel_spmd(
        _built[repeat], in_maps, list(range(NCORES)), trace=trace
    )


def _postprocess(results):
    out = np.empty((B,), np.float32)
    for c in range(NCORES):
        ra = results[c]["rowacc"]  # [BPC, 128, NCHUNK*NSPAN]
        ca = results[c]["colacc"]
        for b in range(BPC):
            m1 = ra[b].reshape(P, NCHUNK, NSPAN).min(axis=2)
            m2 = ca[b].reshape(P, NCHUNK, NSPAN).min(axis=2)
            out[c * BPC + b] = np.float32(
                m1.mean(dtype=np.float64) + m2.mean(dtype=np.float64)
            )
    return out


def kernel(x1: np.ndarray, x2: np.ndarray) -> np.ndarray:
    res = _run(_prep_in_maps(x1, x2))
    return _postprocess(res.results)


# revision 14
# speedup vs baseline: 1580.2828x; 2.0820x over previous
"""Chamfer distance loss kernel for Trainium2 (8 NeuronCores, SPMD).

Problem: x1 [16, 4096, 3], x2 [16, 4096, 3] ->
    chamfer[b] = mean_i min_j ||x1[b,i]-x2[b,j]||^2 + mean_j min_i ||...||^2

Strategy (v3):
  - Data-parallel over batch: 2 batches per core.
  - Distance embedding: u_i = [x1_i, |x1_i|^2, 1], v_j = [-2*x2_j, 1, |x2_j|^2]
    => u_i . v_j = ||x1_i - x2_j||^2; one [K, 128] x [K, FD] matmul produces a
    128 x FD distance block in PSUM.
  - fp32 matmuls stream at 1/4 rate on the PE, so inputs are split into 3
    bfloat16 components and K carries the full 3x3 outer product (K = 45):
    exactly (uh+um+ul).(vh+vm+vl) ~ fp32-accurate at bf16 streaming rate.
  - Single generation per batch serves BOTH reductions: ScalarE copies each
    PSUM tile to fp16 SBUF; VectorE runs 2x-rate fp16 tensor-tensor mins:
      row-min:  binary folds over j then one 1x reduce per i-chunk
      col-min:  elementwise running fold across i-chunks into runmin[128,4096]
    Col-min finishes with PE 128x128 transposes + one batched reduce
    (partition residue -> free axis).
  - Host folds the small [128, 32] slot tensors into the final [16] means.
"""

import sys

for _p in ("/opt/trn_rl_repo",):
    if _p not in sys.path:
        sys.path.insert(0, _p)

import ml_dtypes
import numpy as np

B, N, M = 16, 4096, 4096
NCORES = 8
BPC = B // NCORES  # batches per core
K = 5  # embedding dim; K3 = 3 bf16 splits x 3 = 45 matmul contraction
K3 = 9 * K
P = 128  # partitions
SPAN = 2048  # distance elements per PSUM tile (4 banks)
NSPAN = M // SPAN  # 2 spans per chunk
MMBLK = 512  # matmul free dim (1 PSUM bank)
NCHUNK = N // P  # 32 chunks of the i-side
NTP = SPAN // P  # 16 transpose blocks per runmin tile

_built = {}


def _build_nc(repeat=1):
    import concourse.bacc as bacc
    import concourse.mybir as mybir
    import concourse.tile as tile

    f32 = mybir.dt.float32
    bf16 = mybir.dt.bfloat16
    fp16 = mybir.dt.float16

    nc = bacc.Bacc(
        "TRN2", target_bir_lowering=False, debug=False, num_devices=NCORES
    )
    ut3_ext = nc.dram_tensor("ut3", [BPC, K3, N], bf16, kind="ExternalInput").ap()
    vr3_ext = nc.dram_tensor("vr3", [BPC, K3, M], bf16, kind="ExternalInput").ap()
    ra_ext = nc.dram_tensor(
        "rowacc", [BPC, P, NCHUNK], f32, kind="ExternalOutput"
    ).ap()
    ca_ext = nc.dram_tensor(
        "colacc", [BPC, P, M // P], f32, kind="ExternalOutput"
    ).ap()

    with tile.TileContext(nc) as tc:
        with (
            tc.tile_pool(name="const", bufs=1) as cpool,
            tc.tile_pool(name="uv", bufs=1) as uvpool,
            tc.tile_pool(name="acc", bufs=1) as apool,
            tc.tile_pool(name="work", bufs=4) as wpool,
            tc.tile_pool(name="psum", bufs=2, space="PSUM") as ppool,
        ):
            from concourse import masks

            ident = cpool.tile([P, P], fp16, tag="ident", name="ident")
            masks.make_identity(nc, ident[:])

            def body():
                _body(
                    nc, mybir, uvpool, apool, wpool, ppool, ident,
                    (ut3_ext, vr3_ext), (ra_ext, ca_ext),
                )

            if repeat == 1:
                body()
            else:
                with tc.For_i(0, repeat, 1):
                    body()
    nc.compile()
    return nc


def _body(nc, mybir, uvpool, apool, wpool, ppool, ident, ins, outs):
    f32 = mybir.dt.float32
    bf16 = mybir.dt.bfloat16
    fp16 = mybir.dt.float16
    mn = mybir.AluOpType.min
    X = mybir.AxisListType.X
    ut3_ext, vr3_ext = ins
    ra_ext, ca_ext = outs
    for b in range(BPC):
        Ut = uvpool.tile([K3, N], bf16, tag=f"ut{b}", name=f"ut{b}")
        Vr = uvpool.tile([K3, M], bf16, tag=f"vr{b}", name=f"vr{b}")
        nc.sync.dma_start(Ut[:], ut3_ext[b])
        nc.sync.dma_start(Vr[:], vr3_ext[b])
        rowacc = apool.tile([P, NCHUNK], f32, tag=f"ra{b}", name=f"ra{b}")
        colacc = apool.tile([P, M // P], f32, tag=f"ca{b}", name=f"ca{b}")
        runmin = [
            apool.tile([P, SPAN], fp16, tag=f"rm{b}_{sp}", name=f"rm{b}_{sp}")
            for sp in range(NSPAN)
        ]
        for c in range(NCHUNK):
            cps = []
            for sp in range(NSPAN):
                dist = ppool.tile([P, SPAN], f32, tag="dist", name="dist")
                for h in range(SPAN // MMBLK):
                    j0 = sp * SPAN + h * MMBLK
                    nc.tensor.matmul(
                        dist[:, h * MMBLK : (h + 1) * MMBLK],
                        Ut[:, c * P : (c + 1) * P],
                        Vr[:, j0 : j0 + MMBLK],
                        start=True,
                        stop=True,
                    )
                cp = wpool.tile([P, SPAN], fp16, tag="cp", name="cp")
                nc.scalar.copy(cp[:], dist[:])
                cps.append(cp)
                # col-min: running elementwise fold across i-chunks
                if c == 0:
                    nc.vector.tensor_copy(runmin[sp][:], cp[:])
                else:
                    nc.vector.tensor_tensor(
                        out=runmin[sp][:], in0=cp[:], in1=runmin[sp][:], op=mn
                    )
            # row-min: binary fold over j, then one reduce
            rowf = wpool.tile([P, SPAN], fp16, tag="rowf", name="rowf")
            nc.vector.tensor_tensor(
                out=rowf[:], in0=cps[0][:], in1=cps[1][:], op=mn
            )
            rowf2 = wpool.tile([P, SPAN // 2], fp16, tag="rowf2", name="rowf2")
            nc.vector.tensor_tensor(
                out=rowf2[:], in0=rowf[:, : SPAN // 2], in1=rowf[:, SPAN // 2 :], op=mn
            )
            nc.vector.tensor_reduce(
                out=rowacc[:, c : c + 1], in_=rowf2[:], axis=X, op=mn
            )
        # col-min finalize: partition residue -> free axis via PE transpose
        for sp in range(NSPAN):
            tp = ppool.tile([P, SPAN], fp16, tag="dist", name="tp")
            for t in range(NTP):
                nc.tensor.transpose(
                    tp[:, t * P : (t + 1) * P],
                    runmin[sp][:, t * P : (t + 1) * P],
                    ident[:],
                )
            nc.vector.tensor_reduce(
                out=colacc[:, sp * NTP : (sp + 1) * NTP],
                in_=tp[:].rearrange("p (t x) -> p t x", x=P),
                axis=X,
                op=mn,
            )
        nc.sync.dma_start(ra_ext[b], rowacc[:])
        nc.sync.dma_start(ca_ext[b], colacc[:])


def _split3(a):
    """Split fp32 array into 3 bf16 components summing to ~a (fp32 accurate)."""
    a = np.asarray(a, np.float32)
    h = a.astype(ml_dtypes.bfloat16)
    r = a - h.astype(np.float32)
    m = r.astype(ml_dtypes.bfloat16)
    l = (r - m.astype(np.float32)).astype(ml_dtypes.bfloat16)
    return h, m, l


def _prep_in_maps(x1: np.ndarray, x2: np.ndarray):
    x1 = np.asarray(x1, dtype=np.float32)
    x2 = np.asarray(x2, dtype=np.float32)
    # center the clouds (chamfer is translation invariant; shrinks magnitudes
    # so the bf16-split dot keeps more effective precision)
    ctr = 0.5 * (x1.mean(axis=(1,), keepdims=True) + x2.mean(axis=(1,), keepdims=True))
    x1c = x1 - ctr
    x2c = x2 - ctr
    n1 = (x1c.astype(np.float64) ** 2).sum(-1).astype(np.float32)  # [B, N]
    n2 = (x2c.astype(np.float64) ** 2).sum(-1).astype(np.float32)  # [B, M]
    u_all = np.concatenate(
        [x1c.transpose(0, 2, 1), n1[:, None, :], np.ones((B, 1, N), np.float32)],
        axis=1,
    )  # [B, 5, N]
    v_all = np.concatenate(
        [
            -2.0 * x2c.transpose(0, 2, 1),
            np.ones((B, 1, M), np.float32),
            n2[:, None, :],
        ],
        axis=1,
    )  # [B, 5, M]
    uh, um, ul = _split3(u_all)
    vh, vm, vl = _split3(v_all)
    usplit = np.concatenate([uh, um, ul], axis=1)  # [B, 15, N]
    ut3 = np.tile(usplit, (1, 3, 1))  # [B, 45, N]  (uh um ul) x3
    vr3 = np.concatenate([vh, vh, vh, vm, vm, vm, vl, vl, vl], axis=1)  # [B, 45, M]
    c = np.ascontiguousarray
    return [
        {
            "ut3": c(ut3[i * BPC : (i + 1) * BPC]),
            "vr3": c(vr3[i * BPC : (i + 1) * BPC]),
        }
        for i in range(NCORES)
    ]


def _run(in_maps, trace=False, repeat=1):
    from concourse.bass_utils import run_bass_kernel_spmd

    if repeat not in _built:
        _built[repeat] = _build_nc(repeat)
    return run_bass_kernel_spmd(
        _built[repeat], in_maps, list(range(NCORES)), trace=trace
    )


def _postprocess(results):
    out = np.empty((B,), np.float32)
    for c in range(NCORES):
        ra = results[c]["rowacc"]  # [BPC, 128, NCHUNK]
        ca = results[c]["colacc"]  # [BPC, 128, M//P]
        for b in range(BPC):
            out[c * BPC + b] = np.float32(
                ra[b].mean(dtype=np.float64) + ca[b].mean(dtype=np.float64)
            )
    return out


def kernel(x1: np.ndarray, x2: np.ndarray) -> np.ndarray:
    res = _run(_prep_in_maps(x1, x2))
    return _postprocess(res.results)
